# revision 3
# baseline (speedup 1.0000x reference)
"""Trainium2 Bass kernel for a 2-layer multi-head GAT (nn_MultiHeadGATLayer).

Architecture (hardcoded): N=16384 nodes, D=512, E=540672 edges
(32 random in-edges/node + self loop), layer 1: 8 heads x 64 + elu concat,
layer 2: single 512-dim head + elu, residual.

Distribution: destination-node sharding across 8 cores. Per layer each core
computes z = x @ W for its 2048-node shard, AllGathers the z-table (row =
[z(512) | exp(es)(H) | exp(0.2 es)(H) | 0-pad] bf16, 640 wide), then runs the
edge phase for its own dst nodes with SWDGE row gathers.

Key trick: exp(leaky_relu(es+ed)) == max(exp(es)exp(ed), exp(.2es)exp(.2ed)),
so per-node exponentials are precomputed in the z-phase and the per-edge
attention numerator is two small multiplies + a max on [128, g, H] tiles; no
per-edge transcendentals. Padding rows are all-zero (contribute 0 to both
numerator and denominator). The self-loop slot is loaded with one static DMA
from the core's own zloc shard instead of gather descriptors.
"""
import os
import sys

sys.path.insert(0, "/opt/trn_rl_repo")

import numpy as np
import ml_dtypes

import concourse.bacc as bacc
import concourse.mybir as mybir
from concourse.tile import TileContext
from concourse.bass_utils import run_bass_kernel_spmd
from concourse.library_config import mlp

F32 = mybir.dt.float32
BF16 = mybir.dt.bfloat16
I16 = mybir.dt.int16

B, S, D = 64, 256, 512
H, DO = 8, 64
ALPHA = 0.2
N = B * S
DEG = 32
NCORES = 8
P = 128
SHN = N // NCORES          # nodes per core (2048)
NT = SHN // P              # node tiles per core (16)
ROWW = 640                 # table row width (512 z + 8 e1 + 8 e2 + pad), bf16
DUMMY = N                  # dummy (all-zero) row index for padding slots
CHUNKS1 = [0, 512, 1024, 1536, 2048]         # L1 AG chunk bounds (positions)
CHUNKS2 = [0, 768, 1280, 1664, 1920, 2048]   # L2 AG chunk bounds

_cache = {}


def _build_host(src, dst):
    """Permutation, per-tile slot schedule, padded gather indices.

    Only the appended self-loops (the last N edge entries) are pulled out into
    the static slot; coincidental src==dst pairs in the random part stay in
    the gathered edge list.
    """
    nr = N * DEG
    src_r, dst_r = src[:nr], dst[:nr]
    deg = np.bincount(dst_r, minlength=N)
    order = np.argsort(-deg, kind="stable")
    core_of = np.empty(N, np.int32)
    pos_of = np.empty(N, np.int32)
    for c in range(NCORES):
        nodes_c = order[c::NCORES]
        core_of[nodes_c] = c
        pos_of[nodes_c] = np.arange(SHN)
    nodes = [order[c::NCORES] for c in range(NCORES)]
    # chunk-major table layouts (per layer): AG chunk q covers positions
    # [S[q], S[q+1]) of every core and lands at ztab rows
    # [8*S[q] + c*len_q, ...). L1 uses two big chunks (startup is CC-bound);
    # L2 uses shrinking chunks so the last chunk (after the last edge tile)
    # is small.
    def chunk_tabpos(bounds):
        tp = np.empty(N, np.int64)
        p = pos_of.astype(np.int64)
        for q in range(len(bounds) - 1):
            lo, hi = bounds[q], bounds[q + 1]
            m = (p >= lo) & (p < hi)
            tp[m] = NCORES * lo + core_of[m].astype(np.int64) * (hi - lo) \
                + (p[m] - lo)
        return tp.astype(np.int32)

    tabpos1 = chunk_tabpos(CHUNKS1)
    tabpos2 = chunk_tabpos(CHUNKS2)

    eorder = np.argsort(dst_r, kind="stable")
    src_s = src_r[eorder]
    cum = np.zeros(N + 1, np.int64)
    np.cumsum(deg, out=cum[1:])

    # exact per-tile slot count = max degree over the 8 cores' tile j
    K_sched = np.zeros(NT, np.int64)
    for c in range(NCORES):
        dg = deg[nodes[c]]
        for j in range(NT):
            K_sched[j] = max(K_sched[j], dg[j * P:(j + 1) * P].max())
    # group sizes per tile: full 8s plus remainder
    groups = []
    for j in range(NT):
        k = int(K_sched[j])
        gs = [8] * (k // 8)
        if k % 8:
            gs.append(k % 8)
        groups.append(gs)
    totK = int(K_sched.sum())
    idxw = totK * P // 16

    def build_idx(tabpos):
        idx_cores = []
        for c in range(NCORES):
            blocks = []
            nds = nodes[c]
            for j in range(NT):
                nj = nds[j * P:(j + 1) * P]
                Kj = int(K_sched[j])
                pad = np.full((P, Kj), DUMMY, np.int32)
                for i, n in enumerate(nj):
                    d0 = int(deg[n])
                    pad[i, :d0] = tabpos[src_s[cum[n]:cum[n] + d0]]
                blocks.append(pad.T.reshape(-1))      # slot-major [Kj, P]
            flat = np.concatenate(blocks).astype(np.int32)
            assert flat.max() <= 32767
            w = flat.reshape(-1, 16).T
            idx_cores.append(np.tile(w, (8, 1)).astype(np.int16))
        return idx_cores

    return (nodes, [list(map(int, g)) for g in groups], totK, idxw,
            build_idx(tabpos1), build_idx(tabpos2))


def _build_program(groups, idxw):
    nc = bacc.Bacc("TRN2", target_bir_lowering=False, debug=False,
                   num_devices=NCORES)

    xT_in = nc.dram_tensor("xT", [D, SHN], F32, kind="ExternalInput")
    x_in = nc.dram_tensor("x", [SHN, D], F32, kind="ExternalInput")
    w1_in = nc.dram_tensor("w1", [D, D], F32, kind="ExternalInput")
    w1t_in = nc.dram_tensor("w1t", [D, D], F32, kind="ExternalInput")
    a1_in = nc.dram_tensor("a1", [D, 16], F32, kind="ExternalInput")
    w2_in = nc.dram_tensor("w2", [D, D], F32, kind="ExternalInput")
    w2t_in = nc.dram_tensor("w2t", [D, D], F32, kind="ExternalInput")
    a2_in = nc.dram_tensor("a2", [D, 2], F32, kind="ExternalInput")
    id_in = nc.dram_tensor("ident", [P, P], F32, kind="ExternalInput")
    idx1_in = nc.dram_tensor("idx1", [P, idxw], I16, kind="ExternalInput")
    idx2_in = nc.dram_tensor("idx2", [P, idxw], I16, kind="ExternalInput")
    out = nc.dram_tensor("out", [SHN, D], F32, kind="ExternalOutput")

    zloc1 = nc.dram_tensor("zloc1", [SHN, ROWW], BF16, kind="Internal")
    zloc2 = nc.dram_tensor("zloc2", [SHN, ROWW], BF16, kind="Internal")
    ztab1 = nc.dram_tensor("ztab1", [N + P, ROWW], BF16, kind="Internal",
                           addr_space="Shared")
    ztab2 = nc.dram_tensor("ztab2", [N + P, ROWW], BF16, kind="Internal",
                           addr_space="Shared")

    Exp = mybir.ActivationFunctionType.Exp
    Relu = mybir.ActivationFunctionType.Relu
    Copy = mybir.ActivationFunctionType.Copy
    ADD = mybir.AluOpType.add
    MULT = mybir.AluOpType.mult
    MAX = mybir.AluOpType.max

    with TileContext(nc) as tc:
        with tc.tile_pool(name="const", bufs=1) as cpool, \
             tc.tile_pool(name="work", bufs=2) as wpool, \
             tc.tile_pool(name="ep", bufs=2) as epool, \
             tc.tile_pool(name="gat", bufs=3) as gpool, \
             tc.tile_pool(name="wg", bufs=4) as wgpool, \
             tc.tile_pool(name="pz", bufs=2, space="PSUM") as pzpool, \
             tc.tile_pool(name="pe", bufs=3, space="PSUM") as pepool, \
             tc.tile_pool(name="pt", bufs=1, space="PSUM") as ptpool:

            nc.gpsimd.load_library(mlp)

            # ---------- constants / weights ----------
            identf = cpool.tile([P, P], F32)
            nc.sync.dma_start(identf[:], id_in[:])
            identb = cpool.tile([P, P], BF16)
            nc.vector.tensor_copy(identb[:], identf[:])

            idx1 = cpool.tile([P, idxw], I16)
            nc.sync.dma_start(idx1[:], idx1_in[:])
            idx2 = cpool.tile([P, idxw], I16)
            nc.sync.dma_start(idx2[:], idx2_in[:])

            w1b, w2b, w1tf, w2tf = [], [], [], []
            for cidx in range(4):
                wf = wpool.tile([P, D], F32, tag="wload")
                nc.sync.dma_start(wf[:], w1_in[cidx * P:(cidx + 1) * P, :])
                wb = cpool.tile([P, D], BF16, tag=f"w1b{cidx}")
                nc.vector.tensor_copy(wb[:], wf[:])
                w1b.append(wb)
                wf2 = wpool.tile([P, D], F32, tag="wload")
                nc.sync.dma_start(wf2[:], w2_in[cidx * P:(cidx + 1) * P, :])
                wb2 = cpool.tile([P, D], BF16, tag=f"w2b{cidx}")
                nc.vector.tensor_copy(wb2[:], wf2[:])
                w2b.append(wb2)
                t1w = cpool.tile([P, D], F32, tag=f"w1t{cidx}")
                nc.sync.dma_start(t1w[:], w1t_in[cidx * P:(cidx + 1) * P, :])
                w1tf.append(t1w)
                t2w = cpool.tile([P, D], F32, tag=f"w2t{cidx}")
                nc.sync.dma_start(t2w[:], w2t_in[cidx * P:(cidx + 1) * P, :])
                w2tf.append(t2w)

            a1f = [cpool.tile([P, 16], F32, tag=f"a1f{c}", name=f"a1f{c}") for c in range(4)]
            a2f = [cpool.tile([P, 2], F32, tag=f"a2f{c}", name=f"a2f{c}") for c in range(4)]
            for cidx in range(4):
                nc.sync.dma_start(a1f[cidx][:], a1_in[cidx * P:(cidx + 1) * P, :])
                nc.sync.dma_start(a2f[cidx][:], a2_in[cidx * P:(cidx + 1) * P, :])

            va1, va2 = [], []
            for dchunk in range(4):
                pv = pzpool.tile([P, 16], F32, tag="pzB", bufs=2)
                for fc in range(4):
                    nc.tensor.matmul(pv[:], w1tf[fc][:, dchunk * P:(dchunk + 1) * P],
                                     a1f[fc][:], start=(fc == 0), stop=(fc == 3))
                vb = cpool.tile([P, 16], BF16, tag=f"va1{dchunk}")
                nc.vector.tensor_copy(vb[:], pv[:])
                va1.append(vb)
                pv2 = pzpool.tile([P, 16], F32, tag="pzB", bufs=2)
                for fc in range(4):
                    nc.tensor.matmul(pv2[:, 0:2], w2tf[fc][:, dchunk * P:(dchunk + 1) * P],
                                     a2f[fc][:], start=(fc == 0), stop=(fc == 3))
                vb2 = cpool.tile([P, 2], BF16, tag=f"va2{dchunk}")
                nc.vector.tensor_copy(vb2[:], pv2[:, 0:2])
                va2.append(vb2)

            # dummy (padding) row: all zeros
            drow = cpool.tile([1, ROWW], BF16)
            nc.vector.memset(drow[:], 0.0)
            for ztab in (ztab1, ztab2):
                nc.sync.dma_start(ztab[N:N + 1, :], drow[:])

            # per-dst exp(ed), exp(.2 ed) for both layers
            ed1a = cpool.tile([P, NT, H], BF16)
            ed1b = cpool.tile([P, NT, H], BF16)
            ed2a = cpool.tile([P, NT, 1], BF16)
            ed2b = cpool.tile([P, NT, 1], BF16)
            hcTb = [cpool.tile([P, SHN], BF16, tag=f"hcT{c}", name=f"hcT{c}") for c in range(4)]
            xTb = [cpool.tile([P, SHN], BF16, tag=f"xT{c}", name=f"xT{c}") for c in range(4)]
            for cidx in range(4):
                xf = wpool.tile([P, SHN], F32, tag="xload")
                nc.sync.dma_start(xf[:], xT_in[cidx * P:(cidx + 1) * P, :])
                nc.vector.tensor_copy(xTb[cidx][:], xf[:])

            def z_tile(nt, lhs_blocks, wb, va, zloc, eda, edb, ncols):
                """z-phase for one node tile: matmuls + exp embeds + store."""
                pa = pzpool.tile([P, D], F32, tag="pzA")
                pb = pzpool.tile([P, 16], F32, tag="pzB", bufs=2)
                for cidx in range(4):
                    lb = lhs_blocks[cidx][:, nt * P:(nt + 1) * P]
                    nc.tensor.matmul(pa[:], lb, wb[cidx][:],
                                     start=(cidx == 0), stop=(cidx == 3))
                for cidx in range(4):
                    lb = lhs_blocks[cidx][:, nt * P:(nt + 1) * P]
                    nc.tensor.matmul(pb[:, 0:2 * ncols], lb, va[cidx][:],
                                     start=(cidx == 0), stop=(cidx == 3))
                zrow = wpool.tile([P, ROWW], BF16, tag="zrow")
                nc.vector.tensor_copy(zrow[:, 0:D], pa[:])
                nc.scalar.activation(zrow[:, D:D + ncols], pb[:, 0:ncols], Exp)
                nc.scalar.activation(zrow[:, D + ncols:D + 2 * ncols],
                                     pb[:, 0:ncols], Exp, scale=ALPHA)
                nc.vector.memset(zrow[:, D + 2 * ncols:ROWW], 0.0)
                nc.scalar.activation(eda[:, nt, :], pb[:, ncols:2 * ncols], Exp)
                nc.scalar.activation(edb[:, nt, :], pb[:, ncols:2 * ncols],
                                     Exp, scale=ALPHA)
                nc.sync.dma_start(zloc[nt * P:(nt + 1) * P, :], zrow[:])

            # ---------- layer-1 z phase + chunked AllGather ----------
            for nt in range(NT):
                z_tile(nt, xTb, w1b, va1, zloc1, ed1a, ed1b, H)
                if nt % 4 == 3:
                    q = nt // 4
                    lo, hi = CHUNKS1[q], CHUNKS1[q + 1]
                    nc.gpsimd.collective_compute(
                        "AllGather", mybir.AluOpType.bypass,
                        replica_groups=[list(range(NCORES))],
                        ins=[zloc1[lo:hi, :]],
                        outs=[ztab1[NCORES * lo:NCORES * hi, :]])

            # ---------- edge phase (both layers) ----------
            def edge_tile(nt, ztab, zloc, eda, edb, nheads, idx, idx_off, out_cb):
                rep = D // nheads
                Kj = sum(groups[nt]) + 1          # + self slot
                po = pepool.tile([P, D], F32, tag="pout")
                aa = wpool.tile([P, 64, nheads, 2], BF16, tag="aa")

                def attn(gsrc, k0, g):
                    """a[:, k0:k0+g] = max(e1*ed1, e2*ed2) with dup'd pairs."""
                    e1 = gsrc[:, :, D:D + nheads].unsqueeze(3) \
                        .broadcast_to([P, g, nheads, 2])
                    e2 = gsrc[:, :, D + nheads:D + 2 * nheads].unsqueeze(3) \
                        .broadcast_to([P, g, nheads, 2])
                    d1 = eda[:, nt, :].unsqueeze(1).unsqueeze(3) \
                        .broadcast_to([P, g, nheads, 2])
                    d2 = edb[:, nt, :].unsqueeze(1).unsqueeze(3) \
                        .broadcast_to([P, g, nheads, 2])
                    m1 = wpool.tile([P, 8, nheads, 2], BF16, tag="m1")
                    nc.vector.tensor_tensor(m1[:, 0:g], e1, d1, MULT)
                    m2 = wpool.tile([P, 8, nheads, 2], BF16, tag="m2")
                    nc.vector.tensor_tensor(m2[:, 0:g], e2, d2, MULT)
                    nc.vector.tensor_tensor(aa[:, k0:k0 + g], m1[:, 0:g],
                                            m2[:, 0:g], MAX)

                def accum(gsrc, k0, g):
                    for k in range(g):
                        wg = wgpool.tile([P, D], BF16, tag="wg")
                        g_v = gsrc[:, k, 0:D].rearrange(
                            "p (h r t) -> p h r t", h=nheads, r=rep // 2, t=2)
                        a_v = aa[:, k0 + k, :, :].unsqueeze(2) \
                            .broadcast_to([P, nheads, rep // 2, 2])
                        w_v = wg[:].rearrange(
                            "p (h r t) -> p h r t", h=nheads, r=rep // 2, t=2)
                        nc.vector.tensor_tensor(w_v, g_v, a_v, MULT)
                        kk = k0 + k
                        nc.tensor.matmul(po[:], identb[:], wg[:],
                                         start=(kk == 0), stop=(kk == Kj - 1))

                # static self-loop slot from own shard
                gstat = gpool.tile([P, 1, ROWW], BF16, tag="gstat")
                nc.sync.dma_start(
                    gstat[:].rearrange("p a b -> p (a b)"),
                    zloc[nt * P:(nt + 1) * P, :])
                attn(gstat[:], 0, 1)
                accum(gstat[:], 0, 1)

                k0 = 1
                for g in groups[nt]:
                    gt = gpool.tile([P, 8, ROWW], BF16, tag="G")
                    nidx = P * g
                    nc.gpsimd.dma_gather(
                        gt[:, 0:g, :], ztab[:], idx[:, idx_off:idx_off + nidx // 16],
                        nidx, nidx, ROWW)
                    idx_off += nidx // 16
                    attn(gt[:, 0:g, :], k0, g)
                    accum(gt[:, 0:g, :], k0, g)
                    k0 += g

                den = wpool.tile([P, nheads], F32, tag="den")
                nc.vector.tensor_reduce(
                    den[:], aa[:, 0:Kj].rearrange("p k h t -> p h k t"),
                    mybir.AxisListType.XY, ADD)
                rcp = wpool.tile([P, nheads], F32, tag="rcp")
                nc.vector.reciprocal(rcp[:], den[:])
                t1 = epool.tile([P, D], F32, tag="t1")
                r_v = rcp[:].unsqueeze(2).broadcast_to([P, nheads, rep])
                t_v = t1[:].rearrange("p (h r) -> p h r", h=nheads, r=rep)
                nc.vector.scalar_tensor_tensor(
                    t_v, po[:].rearrange("p (h r) -> p h r", h=nheads, r=rep),
                    2.0, r_v, MULT, MULT)
                out_cb(nt, t1)
                return idx_off

            # layer-1 epilogue: elu -> bf16 -> transpose to hcTb, then z2 tile
            def l1_out(nt, t1):
                em = epool.tile([P, D], F32, tag="em")
                nc.scalar.activation(em[:], t1[:], Relu, scale=-1.0)
                ex = epool.tile([P, D], F32, tag="ex")
                nc.scalar.activation(ex[:], em[:], Exp, scale=-1.0)
                pos = epool.tile([P, D], F32, tag="pos")
                nc.scalar.activation(pos[:], t1[:], Relu)
                hc = epool.tile([P, D], BF16, tag="hc")
                nc.vector.scalar_tensor_tensor(hc[:], ex[:], -1.0, pos[:],
                                               ADD, ADD)
                for cidx in range(4):
                    pt = ptpool.tile([P, P], BF16, tag="ptr")
                    nc.tensor.transpose(pt[:], hc[:, cidx * P:(cidx + 1) * P],
                                        identb[:])
                    nc.scalar.activation(
                        hcTb[cidx][:, nt * P:(nt + 1) * P], pt[:], Copy)
                # layer-2 z for this tile (lhsT columns just written)
                z_tile(nt, hcTb, w2b, va2, zloc2, ed2a, ed2b, 1)
                fire = {5: 0, 9: 1, 12: 2, 14: 3, 15: 4}
                if nt in fire:
                    q = fire[nt]
                    lo, hi = CHUNKS2[q], CHUNKS2[q + 1]
                    nc.gpsimd.collective_compute(
                        "AllGather", mybir.AluOpType.bypass,
                        replica_groups=[list(range(NCORES))],
                        ins=[zloc2[lo:hi, :]],
                        outs=[ztab2[NCORES * lo:NCORES * hi, :]])

            off = 0
            for nt in range(NT):
                off = edge_tile(nt, ztab1, zloc1, ed1a, ed1b, H, idx1, off, l1_out)

            # layer-2 epilogue: elu + residual + store
            def l2_out(nt, t1):
                em = epool.tile([P, D], F32, tag="em")
                nc.scalar.activation(em[:], t1[:], Relu, scale=-1.0)
                ex = epool.tile([P, D], F32, tag="ex")
                nc.scalar.activation(ex[:], em[:], Exp, scale=-1.0)
                pos = epool.tile([P, D], F32, tag="pos")
                nc.scalar.activation(pos[:], t1[:], Relu)
                el = epool.tile([P, D], F32, tag="el")
                nc.vector.scalar_tensor_tensor(el[:], ex[:], -1.0, pos[:],
                                               ADD, ADD)
                xr = epool.tile([P, D], F32, tag="xr")
                nc.sync.dma_start(xr[:], x_in[nt * P:(nt + 1) * P, :])
                ot = epool.tile([P, D], F32, tag="ot")
                nc.vector.tensor_tensor(ot[:], el[:], xr[:], ADD)
                nc.sync.dma_start(out[nt * P:(nt + 1) * P, :], ot[:])

            off = 0
            for nt in range(NT):
                off = edge_tile(nt, ztab2, zloc2, ed2a, ed2b, 1, idx2, off, l2_out)

    nc.compile()
    return nc


def kernel(h, W1, a1, Wout, aout, src, dst):
    h = np.asarray(h, np.float32)
    W1 = np.asarray(W1, np.float32)
    a1 = np.asarray(a1, np.float32)
    Wout = np.asarray(Wout, np.float32)
    aout = np.asarray(aout, np.float32)
    src = np.asarray(src, np.int32)
    dst = np.asarray(dst, np.int32)

    x = h.reshape(N, D)
    nodes, groups, totK, idxw, idx1_cores, idx2_cores = _build_host(src, dst)

    key = (tuple(tuple(g) for g in groups),)
    if key not in _cache:
        _cache[key] = _build_program(groups, idxw)
    nc = _cache[key]

    W1cat = np.ascontiguousarray(W1.transpose(1, 0, 2).reshape(D, D))
    A1 = np.zeros((D, 16), np.float32)
    for hh in range(H):
        A1[hh * DO:(hh + 1) * DO, hh] = a1[hh, :DO]
        A1[hh * DO:(hh + 1) * DO, 8 + hh] = a1[hh, DO:]
    A2 = np.stack([aout[:D], aout[D:]], axis=1).astype(np.float32)
    ident = np.eye(P, dtype=np.float32)

    in_maps = []
    for c in range(NCORES):
        xs = np.ascontiguousarray(x[nodes[c]])
        in_maps.append({
            "xT": np.ascontiguousarray(xs.T),
            "x": xs,
            "w1": W1cat,
            "w1t": np.ascontiguousarray(W1cat.T),
            "a1": A1,
            "w2": Wout,
            "w2t": np.ascontiguousarray(Wout.T),
            "a2": A2,
            "ident": ident,
            "idx1": idx1_cores[c],
            "idx2": idx2_cores[c],
        })

    trace = bool(int(os.environ.get("GAT_TRACE", "0")))
    res = run_bass_kernel_spmd(nc, in_maps, core_ids=list(range(NCORES)),
                               trace=trace)
    if trace:
        print("HW exec time:", res.exec_time_ns, "ns")
        print("trace:", res.instructions_and_trace[1]
              if res.instructions_and_trace else None)
    outf = np.zeros((N, D), np.float32)
    for c in range(NCORES):
        outf[nodes[c]] = res.results[c]["out"]
    return outf.reshape(B, S, D)
